# revision 1
# baseline (speedup 1.0000x reference)
"""AttnBlock (GroupNorm -> QKV -> 4096x4096 spatial attention -> proj -> residual)
for Trainium2, sharded over 8 NeuronCores.

Sharding: core = (batch b, query-slice s); b = core//4, s = core%4.
Each core computes GroupNorm stats + K/V for its full batch image (redundant
across the 4 cores of a batch) and attention/projection for its 1024-query
slice. No collectives.

Device layouts (per core):
  hn, q, k: [c, i] with c on partitions (4 chunks of 128)
  vT:       [j, c] with j on partitions (16 tiles of [128, 512] per half)
  scores^T: [j, i] -> softmax along partition axis j:
            exp via ACT (no max subtraction; |scores| <= ~6 by construction),
            denominator via ones-vector matmul, applied after the output
            projection (division commutes with the channel contraction).
All matmuls run as float32r (tf32-like, full PE rate at N=512).
"""
import numpy as np
import concourse.bacc as bacc
import concourse.tile as tile
import concourse.mybir as mybir
from concourse.bass_utils import run_bass_kernel_spmd

F32 = mybir.dt.float32
F32R = mybir.dt.float32r
AF = mybir.ActivationFunctionType
OP = mybir.AluOpType

B, C, H, W = 2, 512, 64, 64
HW = H * W                    # 4096
NCORES = 8
NSLICE = 4                    # query slices per batch
SL = HW // NSLICE             # 1024 query positions per core
NG = 32                       # groups
EPS = 1e-6
CCH = C // 128                # 4 channel chunks
NHALF = 2                     # j halves
JH = HW // NHALF              # 2048 j per half
JB = JH // 512                # 4 j-blocks of 512 per half
JC = JH // 128                # 16 j-chunks of 128 per half
IB = SL // 512                # 2 i-blocks of 512
SCALE = float(C) ** -0.5


def build(reps: int = 1):
    nc = bacc.Bacc("TRN2", target_bir_lowering=False)
    dr = {}
    dr["xf"] = nc.dram_tensor("xf", [C, HW], F32, kind="ExternalInput")
    dr["xs"] = nc.dram_tensor("xs", [C, SL], F32, kind="ExternalInput")
    dr["wqT"] = nc.dram_tensor("wqT", [C, C], F32, kind="ExternalInput")
    dr["wkT"] = nc.dram_tensor("wkT", [C, C], F32, kind="ExternalInput")
    dr["wvT"] = nc.dram_tensor("wvT", [C, C], F32, kind="ExternalInput")
    dr["woT"] = nc.dram_tensor("woT", [C, C], F32, kind="ExternalInput")
    # packed per-channel vectors: [chunk, partition, {bq, bk, bo2, gamma, beta}]
    dr["ball"] = nc.dram_tensor("ball", [CCH, 128, 5], F32, kind="ExternalInput")
    dr["gavg"] = nc.dram_tensor("gavg", [128, 128], F32, kind="ExternalInput")
    dr["y"] = nc.dram_tensor("y", [C, SL], F32, kind="ExternalOutput")

    with tile.TileContext(nc) as tc:
        _body(nc, tc, reps, dr)
    nc.finalize()
    return nc


def _body(nc, tc, reps, dr):
    from contextlib import ExitStack
    with ExitStack() as ctx:
        pw = ctx.enter_context(tc.tile_pool(name="pw", bufs=1))
        pc = ctx.enter_context(tc.tile_pool(name="pc", bufs=1))
        pq = ctx.enter_context(tc.tile_pool(name="pq", bufs=1))
        pio = ctx.enter_context(tc.tile_pool(name="pio", bufs=1))
        pdr = ctx.enter_context(tc.tile_pool(name="pdr", bufs=2, space="DRAM"))
        pmm = ctx.enter_context(tc.tile_pool(name="pmm", bufs=2, space="PSUM"))
        patt = ctx.enter_context(tc.tile_pool(name="patt", bufs=1, space="PSUM"))

        # small constants needed by phase A (emitted first: x-chunk DMAs and
        # groupnorm stats are the critical path at kernel start)
        ball_t = pc.tile([128, CCH, 5], F32, tag="ball", name="ball")
        nc.sync.dma_start(
            out=ball_t,
            in_=bacc.bass.AP(tensor=dr["ball"], offset=0,
                             ap=[[5, 128], [128 * 5, CCH], [1, 5]]))
        bq_t = [ball_t[:, c, 0:1] for c in range(CCH)]
        bk_t = [ball_t[:, c, 1:2] for c in range(CCH)]
        bo_t = [ball_t[:, c, 2:3] for c in range(CCH)]
        gm_t = [ball_t[:, c, 3:4] for c in range(CCH)]
        bt_t = [ball_t[:, c, 4:5] for c in range(CCH)]
        gav_t = pw.tile([128, 128], F32R, tag="gav", name="gav")
        nc.sync.dma_start(out=gav_t, in_=dr["gavg"][:, :].bitcast(F32R))

        epst = pc.tile([128, 1], F32, tag="epst", name="epst")
        nc.vector.memset(epst, EPS)
        onesf = pc.tile([128, 1], F32, tag="onesf", name="onesf")
        nc.vector.memset(onesf, 1.0)
        ones_r = pc.tile([128, 1], F32R, tag="onesr", name="onesr")
        nc.vector.tensor_copy(ones_r[:, :], onesf[:, :])

        # weight tiles declared up front, loaded late in _attn_once so the
        # x-stats DMAs win the queue at kernel start
        wk_t = [pw.tile([128, C], F32R, tag=f"wk{c}", name=f"wk{c}") for c in range(CCH)]
        wv_t = [pw.tile([128, C], F32R, tag=f"wv{c}", name=f"wv{c}") for c in range(CCH)]
        wo_t = [pw.tile([128, C], F32R, tag=f"wo{c}", name=f"wo{c}") for c in range(CCH)]

        consts = dict(wk_t=wk_t, wv_t=wv_t, wo_t=wo_t, gav_t=gav_t,
                      bq_t=bq_t, bk_t=bk_t, bo_t=bo_t, gm_t=gm_t, bt_t=bt_t,
                      ones_r=ones_r, epst=epst, w_loaded=False)
        for _ in range(reps):
            _attn_once(nc, tc, pw, pc, pq, pio, pmm, patt, pdr, dr, consts)
            consts["w_loaded"] = True


def _attn_once(nc, tc, pw, pc, pq, pio, pmm, patt, pdr, dr, cst):
    xf, xs, y = dr["xf"], dr["xs"], dr["y"]
    wk_t, wv_t, wo_t = cst["wk_t"], cst["wv_t"], cst["wo_t"]
    bq_t, bk_t, bo_t = cst["bq_t"], cst["bk_t"], cst["bo_t"]
    gm_t, bt_t = cst["gm_t"], cst["bt_t"]
    gav_t, ones_r, epst = cst["gav_t"], cst["ones_r"], cst["epst"]

    # ---- Phase A: groupnorm stats -> per-channel A (scale), B (shift) ----
    A_t = [pc.tile([128, 1], F32, tag=f"A{c}", name=f"A{c}") for c in range(CCH)]
    B_t = [pc.tile([128, 1], F32, tag=f"Bb{c}", name=f"Bb{c}") for c in range(CCH)]
    with tc.tile_pool(name="pstat", bufs=2) as pstat:
        for c in range(CCH):
            cs = slice(c * 128, (c + 1) * 128)
            xc = pstat.tile([128, HW], F32, tag="xstat", name="xstat")
            nc.sync.dma_start(out=xc, in_=xf[cs, :])
            st = pio.tile([128, 8, 6], F32, tag="st", name="st", bufs=2)
            for s8 in range(8):
                nc.vector.bn_stats(out=st[:, s8, :],
                                   in_=xc[:, s8 * 512:(s8 + 1) * 512])
            mv = pio.tile([128, 2], F32, tag="mv", name="mv", bufs=2)
            nc.vector.bn_aggr(out=mv[:, :], in_=st[:, :, :])
            # ex2 = [E[x], E[x^2]] per partition
            ex2 = pio.tile([128, 2], F32, tag="ex2", name="ex2", bufs=2)
            nc.vector.tensor_copy(ex2[:, 0:1], mv[:, 0:1])
            nc.vector.tensor_mul(ex2[:, 1:2], mv[:, 0:1], mv[:, 0:1])
            nc.vector.tensor_add(ex2[:, 1:2], ex2[:, 1:2], mv[:, 1:2])
            ex2r = pio.tile([128, 2], F32R, tag="ex2r", name="ex2r", bufs=2)
            nc.vector.tensor_copy(ex2r[:, :], ex2[:, :])
            # group-average + broadcast back to channels via averaging matrix
            bc_ps = pmm.tile([128, 2], F32, tag="mm", name="mm")
            nc.tensor.matmul(bc_ps[:, :], gav_t[:, :], ex2r[:, :],
                             start=True, stop=True)
            bcs = pio.tile([128, 2], F32, tag="bcs", name="bcs", bufs=2)
            nc.vector.tensor_copy(bcs[:, :], bc_ps[:, :])
            # var = E[x^2]-mean^2; rstd = 1/sqrt(var+eps); A = rstd*gamma;
            # B = beta - mean*A
            tmp = pio.tile([128, 1], F32, tag="tmp", name="tmp", bufs=2)
            nc.vector.tensor_mul(tmp[:, :], bcs[:, 0:1], bcs[:, 0:1])
            var = pio.tile([128, 1], F32, tag="var", name="var", bufs=2)
            nc.vector.tensor_sub(var[:, :], bcs[:, 1:2], tmp[:, :])
            nc.scalar.activation(var[:, :], var[:, :], AF.Sqrt,
                                 bias=epst[:, :], scale=1.0)
            nc.vector.reciprocal(var[:, :], var[:, :])
            nc.vector.tensor_mul(A_t[c][:, :], var[:, :], gm_t[c])
            nc.vector.tensor_mul(tmp[:, :], bcs[:, 0:1], A_t[c][:, :])
            nc.vector.tensor_sub(B_t[c][:, :], bt_t[c], tmp[:, :])

    # ---- Phase A2: hn slice + Q projection (transient pool) ----
    q_t = [pq.tile([128, SL], F32R, tag=f"q{c}", name=f"q{c}") for c in range(CCH)]
    with tc.tile_pool(name="phns", bufs=1) as phns:
        wq_t = [phns.tile([128, C], F32R, tag=f"wq{c}", name=f"wq{c}")
                for c in range(CCH)]
        for c in range(CCH):
            cs = slice(c * 128, (c + 1) * 128)
            nc.sync.dma_start(out=wq_t[c], in_=dr["wqT"][cs, :].bitcast(F32R))
        hns = [phns.tile([128, SL], F32R, tag=f"hns{c}", name=f"hns{c}")
               for c in range(CCH)]
        for c in range(CCH):
            cs = slice(c * 128, (c + 1) * 128)
            xst = pio.tile([128, SL], F32, tag="xs", name="xs", bufs=2)
            nc.sync.dma_start(out=xst, in_=xs[cs, :])
            nc.vector.tensor_scalar(
                out=hns[c][:, :], in0=xst[:, :],
                scalar1=A_t[c], scalar2=B_t[c], op0=OP.mult, op1=OP.add)
        for ib in range(IB):
            isl = slice(ib * 512, (ib + 1) * 512)
            for co in range(CCH):
                qp = pmm.tile([128, 512], F32, tag="mm", name="mm")
                for ci in range(CCH):
                    nc.tensor.matmul(
                        qp[:, :], wq_t[ci][:, co * 128:(co + 1) * 128],
                        hns[ci][:, isl], start=(ci == 0), stop=(ci == CCH - 1))
                nc.vector.tensor_scalar(
                    out=q_t[co][:, isl], in0=qp[:, :],
                    scalar1=bq_t[co], scalar2=None, op0=OP.add)

    # K/V/O weight loads (after the stats critical path)
    if not cst["w_loaded"]:
        for c in range(CCH):
            cs = slice(c * 128, (c + 1) * 128)
            nc.sync.dma_start(out=wk_t[c], in_=dr["wkT"][cs, :].bitcast(F32R))
            nc.sync.dma_start(out=wv_t[c], in_=dr["wvT"][cs, :].bitcast(F32R))
            nc.sync.dma_start(out=wo_t[c], in_=dr["woT"][cs, :].bitcast(F32R))

    # ---- main pools for K/V + attention ----
    with tc.tile_pool(name="pkv", bufs=1) as pkv, \
         tc.tile_pool(name="pacc", bufs=1) as pacc:
        acc_t = [[pacc.tile([128, 512], F32R, tag=f"acc{ib}_{co}",
                            name=f"acc{ib}_{co}") for co in range(CCH)]
                 for ib in range(IB)]
        den_t = [pacc.tile([1, 512], F32, tag=f"den{ib}", name=f"den{ib}")
                 for ib in range(IB)]
        k_t = [pkv.tile([128, JH], F32R, tag=f"k{c}", name=f"k{c}")
               for c in range(CCH)]
        vt_t = [pkv.tile([128, 512], F32R, tag=f"vt{j}", name=f"vt{j}")
                for j in range(JC)]

        def kv_production(h):
            for jb in range(JB):
                hnb = []
                for ci in range(CCH):
                    cs = slice(ci * 128, (ci + 1) * 128)
                    jsl = slice(h * JH + jb * 512, h * JH + (jb + 1) * 512)
                    xb = pio.tile([128, 512], F32, tag=f"xb{ci}", name="xb", bufs=2)
                    nc.sync.dma_start(out=xb, in_=xf[cs, jsl])
                    hb = pio.tile([128, 512], F32R, tag=f"hnb{ci}", name="hnb")
                    nc.vector.tensor_scalar(
                        out=hb[:, :], in0=xb[:, :],
                        scalar1=A_t[ci], scalar2=B_t[ci], op0=OP.mult, op1=OP.add)
                    hnb.append(hb)
                lsl = slice(jb * 512, (jb + 1) * 512)
                for co in range(CCH):
                    kp = pmm.tile([128, 512], F32, tag="mm", name="mm")
                    for ci in range(CCH):
                        nc.tensor.matmul(
                            kp[:, :], wk_t[ci][:, co * 128:(co + 1) * 128],
                            hnb[ci][:, :], start=(ci == 0), stop=(ci == CCH - 1))
                    nc.vector.tensor_scalar(
                        out=k_t[co][:, lsl], in0=kp[:, :],
                        scalar1=bk_t[co], scalar2=None, op0=OP.add)
                for jt in range(4):
                    vp = pmm.tile([128, 512], F32, tag="mm", name="mm")
                    for ci in range(CCH):
                        nc.tensor.matmul(
                            vp[:, :], hnb[ci][:, jt * 128:(jt + 1) * 128],
                            wv_t[ci][:, :], start=(ci == 0), stop=(ci == CCH - 1))
                    nc.vector.tensor_copy(vt_t[jb * 4 + jt][:, :], vp[:, :])

        def attention(h, ib):
            isl = slice(ib * 512, (ib + 1) * 512)
            att_ps = [patt.tile([128, 512], F32, tag=f"att{co}",
                                name=f"att{co}") for co in range(CCH)]
            den_ps = patt.tile([1, 512], F32, tag="den", name="den")
            for jc in range(JC):
                sp = pmm.tile([128, 512], F32, tag="mm", name="mm")
                for ci in range(CCH):
                    nc.tensor.matmul(
                        sp[:, :], k_t[ci][:, jc * 128:(jc + 1) * 128],
                        q_t[ci][:, isl], start=(ci == 0), stop=(ci == CCH - 1))
                eT = pio.tile([128, 512], F32R, tag="eT", name="eT", bufs=3)
                nc.scalar.activation(eT[:, :], sp[:, :], AF.Exp,
                                     bias=0.0, scale=SCALE)
                for co in range(CCH):
                    nc.tensor.matmul(
                        att_ps[co][:, :], vt_t[jc][:, co * 128:(co + 1) * 128],
                        eT[:, :], start=(jc == 0), stop=(jc == JC - 1))
                nc.tensor.matmul(
                    den_ps[0:1, :], ones_r[:, 0:1], eT[:, :],
                    start=(jc == 0), stop=(jc == JC - 1))
            for co in range(CCH):
                if h == 0:
                    nc.vector.tensor_copy(acc_t[ib][co][:, :], att_ps[co][:, :])
                else:
                    nc.vector.tensor_add(acc_t[ib][co][:, :],
                                         acc_t[ib][co][:, :].bitcast(F32),
                                         att_ps[co][:, :])
            if h == 0:
                nc.vector.tensor_copy(den_t[ib][0:1, :], den_ps[0:1, :])
            else:
                nc.vector.tensor_add(den_t[ib][0:1, :], den_t[ib][0:1, :],
                                     den_ps[0:1, :])

        def finalize(ib):
            # proj first (no dependency on the denominator), divide afterwards
            isl = slice(ib * 512, (ib + 1) * 512)
            rec = pio.tile([1, 512], F32, tag="rec", name="rec", bufs=2)
            nc.vector.reciprocal(rec[0:1, :], den_t[ib][0:1, :])
            rdr = pdr.tile([1, 512], F32, tag="rdr", name="rdr")
            nc.sync.dma_start(out=rdr[0:1, :], in_=rec[0:1, :])
            rec_bc = pio.tile([128, 512], F32, tag="recbc", name="recbc", bufs=2)
            nc.sync.dma_start(out=rec_bc, in_=rdr[0:1, :].partition_broadcast(128))
            for co in range(CCH):
                cs = slice(co * 128, (co + 1) * 128)
                pp = pmm.tile([128, 512], F32, tag="mm", name="mm")
                for ci in range(CCH):
                    nc.tensor.matmul(
                        pp[:, :], wo_t[ci][:, co * 128:(co + 1) * 128],
                        acc_t[ib][ci][:, :], start=(ci == 0), stop=(ci == CCH - 1))
                fin = pio.tile([128, 512], F32, tag="fin", name="fin", bufs=2)
                nc.vector.tensor_mul(fin[:, :], pp[:, :], rec_bc[:, :])
                nc.vector.tensor_scalar(
                    out=fin[:, :], in0=fin[:, :],
                    scalar1=bo_t[co], scalar2=None, op0=OP.add)
                xr = pio.tile([128, 512], F32, tag="xs", name="xr", bufs=2)
                nc.sync.dma_start(out=xr, in_=xs[cs, isl])
                nc.vector.tensor_add(fin[:, :], fin[:, :], xr[:, :])
                nc.sync.dma_start(out=y[cs, isl], in_=fin[:, :])

        kv_production(0)
        attention(0, 0)
        attention(0, 1)
        kv_production(1)
        attention(1, 0)
        finalize(0)
        attention(1, 1)
        finalize(1)


_NC_CACHE = {}


def _get_nc(reps: int = 1):
    if reps not in _NC_CACHE:
        _NC_CACHE[reps] = build(reps)
    return _NC_CACHE[reps]


def _host_inputs(x, norm_gamma, norm_beta, wq, bq, wk, bk, wv, bv, wo, bo):
    f32 = np.float32
    wqT = np.ascontiguousarray(np.asarray(wq, f32).T)
    wkT = np.ascontiguousarray(np.asarray(wk, f32).T)
    wvT = np.ascontiguousarray(np.asarray(wv, f32).T)
    woT = np.ascontiguousarray(np.asarray(wo, f32).T)
    bo2 = np.asarray(bo, f32) + np.asarray(wo, f32) @ np.asarray(bv, f32)
    ball = np.stack([
        np.asarray(bq, f32), np.asarray(bk, f32), bo2,
        np.asarray(norm_gamma, f32), np.asarray(norm_beta, f32),
    ], axis=1).reshape(CCH, 128, 5)
    gavg = (np.kron(np.eye(8, dtype=f32), np.ones((16, 16), f32)) / 16.0)
    shared = {
        "wqT": wqT, "wkT": wkT, "wvT": wvT, "woT": woT,
        "ball": np.ascontiguousarray(ball),
        "gavg": np.ascontiguousarray(gavg, f32),
    }
    x = np.asarray(x, f32)
    in_maps = []
    for core in range(NCORES):
        b, s = core // NSLICE, core % NSLICE
        xfb = np.ascontiguousarray(x[b].reshape(C, HW))
        xsb = np.ascontiguousarray(xfb[:, s * SL:(s + 1) * SL])
        in_maps.append(dict(shared, xf=xfb, xs=xsb))
    return in_maps


def kernel(x, norm_gamma, norm_beta, wq, bq, wk, bk, wv, bv, wo, bo,
           reps: int = 1):
    nc = _get_nc(reps)
    in_maps = _host_inputs(x, norm_gamma, norm_beta, wq, bq, wk, bk, wv, bv,
                           wo, bo)
    res = run_bass_kernel_spmd(nc, in_maps, core_ids=list(range(NCORES)),
                               trace=False)
    out = np.empty((B, C, HW), np.float32)
    for core in range(NCORES):
        b, s = core // NSLICE, core % NSLICE
        out[b][:, s * SL:(s + 1) * SL] = res.results[core]["y"]
    return out.reshape(B, C, H, W)
